# revision 5
# baseline (speedup 1.0000x reference)
"""Trainium2 Bass kernel for nn_CrossAttn_65214783422649.

Algebraic reduction: softmax over R followed by mean over R is identically 1/R,
so the attention branch (Wq, Wk, energy, softmax) cancels:

    sims[i, c] = (a_c + b_i) . cs_c / (||a_c + b_i|| * ||cs_c||)
      a_c  = (gamma/R) * sum_t mask * leaky(cap_c @ Wvt.T + bvt)
      b_i  = mean_r leaky(img_i @ Wvi.T + bvi)
      cs_c = masked-sum_t cap_c          (the /lens cancels inside l2norm)

Sharding: captions 8-way (binpacked by lens so the packed-token tile count is
minimal), images 8-way for b, with a bf16 AllGather of b shards (+ 0.5|b|^2).

All transposed operands (imgT, capT, WviT, WvtT) are prepared host-side (no
device transposes outside the tiny assembly).  Matmul operands are bf16 (half
DMA, full PE rate, FWL, no fp32 LOW_HIGH double-pumping); PSUM accumulation is
fp32.  The img phase is ordered strictly before the caption phase on the PE so
the AllGather triggers as early as possible and overlaps the caption phase.
"""

import numpy as np
import ml_dtypes

import concourse.bass as bass
import concourse.mybir as mybir
import concourse.tile as tile
from concourse import bacc
from concourse.bass import ds, ts
from concourse.bass_utils import run_bass_kernel_spmd
from concourse.tile import add_dep_helper

F32 = mybir.dt.float32
BF16 = mybir.dt.bfloat16
AF = mybir.ActivationFunctionType

N_CORES = 8
B_I, B_C, R, T, D = 128, 128, 36, 64, 1024
C_SH = B_C // N_CORES          # 16 captions per core
I_SH = B_I // N_CORES          # 16 images per core
IMG_TOK = I_SH * R             # 576 image tokens per core
IMG_PAD = 640                  # padded to 5 * 128
KT = D // 128                  # 8 contraction tiles
IT = IMG_PAD // 128            # 5 image token tiles
NEG_SLOPE = 0.1
AGW = D + 1                    # AllGather row width: b row + 0.5*|b|^2

_CACHE: dict = {}


def _build(CT: int, with_bias: bool):
    """CT = number of 128-token caption tiles after host packing."""
    CAP_TOK = CT * 128
    nc = bacc.Bacc("TRN2", target_bir_lowering=False, debug=False,
                   num_devices=N_CORES)

    imgT_d = nc.dram_tensor("imgT", [D, IMG_PAD], BF16, kind="ExternalInput")
    wviT_d = nc.dram_tensor("wviT", [D, D], BF16, kind="ExternalInput")
    capT_d = nc.dram_tensor("capT", [D, CAP_TOK], BF16, kind="ExternalInput")
    wvtT_d = nc.dram_tensor("wvtT", [D, D], BF16, kind="ExternalInput")
    cap_d = nc.dram_tensor("cap", [CAP_TOK, D], BF16, kind="ExternalInput")
    om_a_d = nc.dram_tensor("om_a", [CAP_TOK, C_SH], BF16, kind="ExternalInput")
    om_b_d = nc.dram_tensor("om_b", [IMG_PAD, I_SH], BF16, kind="ExternalInput")
    idb_d = nc.dram_tensor("idb", [128, 128], BF16, kind="ExternalInput")
    gam_d = nc.dram_tensor("gam16", [C_SH, 1], F32, kind="ExternalInput")
    if with_bias:
        bft_d = nc.dram_tensor("bias_vt", [128, D], F32, kind="ExternalInput")
        bfi_d = nc.dram_tensor("bias_vi", [128, D], F32, kind="ExternalInput")
    sims_d = nc.dram_tensor("sims", [C_SH, B_I], F32, kind="ExternalOutput")

    with tile.TileContext(nc) as tc:
        with (
            tc.tile_pool(name="const", bufs=1) as const,
            tc.tile_pool(name="wt", bufs=1) as wtp,
            tc.tile_pool(name="xt", bufs=1) as xtp,
            tc.tile_pool(name="vtx", bufs=3) as vtxp,
            tc.tile_pool(name="gpool", bufs=1) as gp,
            tc.tile_pool(name="small", bufs=1) as sp,
            tc.tile_pool(name="ps_tr", bufs=2, space="PSUM") as ps_tr,
            tc.tile_pool(name="ps_mm", bufs=2, space="PSUM") as ps_mm,
            tc.tile_pool(name="ps_acc", bufs=1, space="PSUM") as ps_acc,
            tc.tile_pool(name="dram", bufs=1, space="DRAM") as dram,
        ):
            # ---- img-phase loads (first: the img phase gates the AllGather)
            om_b_s = const.tile([128, IT, I_SH], BF16, tag="om_b")
            nc.sync.dma_start(
                out=om_b_s[:], in_=om_b_d.rearrange("(a p) c -> p a c", p=128))
            imgT_s = xtp.tile([128, KT, IMG_PAD], BF16, tag="imgT")
            wvi_s = wtp.tile([128, KT, D], BF16, tag="wt_vi")
            for k in range(KT):
                nc.sync.dma_start(out=imgT_s[:, k, :], in_=imgT_d[ts(k, 128), :])
                nc.sync.dma_start(out=wvi_s[:, k, 0:512],
                                  in_=wviT_d[ts(k, 128), 0:512])
            for k in range(KT):
                nc.sync.dma_start(out=wvi_s[:, k, 512:1024],
                                  in_=wviT_d[ts(k, 128), 512:1024])
            if with_bias:
                bias_vi = const.tile([128, D], F32, tag="bias_vi")
                nc.sync.dma_start(out=bias_vi[:], in_=bfi_d[:, :])
                bias_vt = const.tile([128, D], F32, tag="bias_vt")
                nc.sync.dma_start(out=bias_vt[:], in_=bft_d[:, :])

            identb = const.tile([128, 128], BF16, tag="idb")
            nc.sync.dma_start(out=identb[:], in_=idb_d[:, :])
            gam16 = const.tile([C_SH, 1], F32, tag="gam16")
            nc.sync.dma_start(out=gam16[:], in_=gam_d[:, :])
            ones_row = const.tile([1, 128], BF16, tag="ones_row")
            nc.vector.memset(ones_row[:], 1.0)
            ones_col = const.tile([128, 1], BF16, tag="ones_col")
            nc.vector.memset(ones_col[:], 1.0)

            # ---- img matmul phase -> b shard + 0.5*|b|^2, AllGather
            ps_b = [ps_acc.tile([I_SH, 512], F32, tag=f"acc{dh}", name=f"ps_b{dh}")
                    for dh in range(2)]
            for it in range(IT):
                vimg = vtxp.tile([128, D], BF16, tag="vtx", name=f"vimg{it}")
                for dh in range(2):
                    pm = ps_mm.tile([128, 512], F32, tag="mm", name=f"pmi{it}{dh}")
                    for k in range(KT):
                        nc.tensor.matmul(pm[:], imgT_s[:, k, ts(it, 128)],
                                         wvi_s[:, k, ds(dh * 512, 512)],
                                         start=(k == 0), stop=(k == KT - 1))
                    if with_bias:
                        nc.vector.tensor_add(pm[:], pm[:],
                                             bias_vi[:, ds(dh * 512, 512)])
                    nc.scalar.activation(vimg[:, ds(dh * 512, 512)], pm[:],
                                         AF.Prelu, alpha=NEG_SLOPE)
                    nc.tensor.matmul(ps_b[dh][:], om_b_s[:, it, :],
                                     vimg[:, ds(dh * 512, 512)],
                                     start=(it == 0), stop=(it == IT - 1))
            bnat = sp.tile([I_SH, AGW], BF16, tag="bnat")
            for dh in range(2):
                nc.scalar.activation(bnat[:, ds(dh * 512, 512)], ps_b[dh][:],
                                     AF.Identity, scale=1.0 / R)
            bsq_sh = sp.tile([I_SH, D], F32, tag="bsq_sh")
            nc.vector.tensor_mul(bsq_sh[:], bnat[:, 0:D], bnat[:, 0:D])
            nbcol_l = sp.tile([I_SH, 1], F32, tag="nbcol_l")
            nc.vector.reduce_sum(nbcol_l[:], bsq_sh[:], axis=mybir.AxisListType.X)
            nc.vector.tensor_scalar_mul(bnat[:, D:D + 1], nbcol_l[:], 0.5)
            ag_in = dram.tile([I_SH, AGW], BF16, tag="ag_in")
            ag_out = dram.tile([B_I, AGW], BF16, addr_space="Shared", tag="ag_out")
            ag_dma = nc.sync.dma_start(out=ag_in[:], in_=bnat[:])
            nc.gpsimd.collective_compute(
                "AllGather",
                mybir.AluOpType.bypass,
                replica_groups=[list(range(N_CORES))],
                ins=[ag_in[:].opt()],
                outs=[ag_out[:].opt()],
            )

            # ---- cap-phase loads (issued after img loads; overlap img compute)
            om_a_s = const.tile([128, CT, C_SH], BF16, tag="om_a")
            nc.sync.dma_start(
                out=om_a_s[:], in_=om_a_d.rearrange("(a p) c -> p a c", p=128))
            capT_s = xtp.tile([128, KT, CAP_TOK], BF16, tag="capT")
            wvt_s = wtp.tile([128, KT, D], BF16, tag="wt_vt")
            for k in range(KT):
                nc.sync.dma_start(out=capT_s[:, k, :], in_=capT_d[ts(k, 128), :])
                nc.sync.dma_start(out=wvt_s[:, k, :], in_=wvtT_d[ts(k, 128), :])
            cnats = []
            for ct in range(CT):
                cnat = xtp.tile([128, D], BF16, tag=f"cap{ct}", name=f"cnat{ct}")
                nc.sync.dma_start(out=cnat[:], in_=cap_d[ts(ct, 128), :])
                cnats.append(cnat)

            # ---- cap matmul phase -> a, capsum
            ps_a = [ps_acc.tile([C_SH, 512], F32, tag=f"acc{dh}",
                                name=f"ps_a{dh}")[:] for dh in range(2)]
            ps_cm = [ps_acc.tile([C_SH, 512], F32, tag=f"acc{dh+2}",
                                 name=f"ps_cm{dh}")[:] for dh in range(2)]
            first_cap_mm = None
            first_cap_act = None
            for ct in range(CT):
                vtxt = vtxp.tile([128, D], BF16, tag="vtx", name=f"vtxt{ct}")
                for dh in range(2):
                    pm = ps_mm.tile([128, 512], F32, tag="mm", name=f"pmc{ct}{dh}")
                    for k in range(KT):
                        mm = nc.tensor.matmul(pm[:], capT_s[:, k, ts(ct, 128)],
                                              wvt_s[:, k, ds(dh * 512, 512)],
                                              start=(k == 0), stop=(k == KT - 1))
                        if first_cap_mm is None:
                            first_cap_mm = mm
                    if with_bias:
                        nc.vector.tensor_add(pm[:], pm[:],
                                             bias_vt[:, ds(dh * 512, 512)])
                    act = nc.scalar.activation(vtxt[:, ds(dh * 512, 512)], pm[:],
                                               AF.Prelu, alpha=NEG_SLOPE)
                    if first_cap_act is None:
                        first_cap_act = act
                    nc.tensor.matmul(ps_a[dh], om_a_s[:, ct, :],
                                     vtxt[:, ds(dh * 512, 512)],
                                     start=(ct == 0), stop=(ct == CT - 1))
                    nc.tensor.matmul(ps_cm[dh], om_a_s[:, ct, :],
                                     cnats[ct][:, ds(dh * 512, 512)],
                                     start=(ct == 0), stop=(ct == CT - 1))
            # schedule the whole img phase + AllGather kickoff before cap work
            add_dep_helper(first_cap_mm.ins, ag_dma.ins, sync=False,
                           reason="cap matmuls after AllGather kickoff")
            add_dep_helper(first_cap_act.ins, ag_dma.ins, sync=False,
                           reason="cap evacs after AllGather kickoff")

            a_s = sp.tile([C_SH, D], BF16, tag="a_s")
            cs_s = sp.tile([C_SH, D], BF16, tag="cs_s")
            for dh in range(2):
                nc.scalar.activation(a_s[:, ds(dh * 512, 512)], ps_a[dh],
                                     AF.Identity, scale=gam16[:])
                nc.scalar.activation(cs_s[:, ds(dh * 512, 512)], ps_cm[dh],
                                     AF.Copy)

            # ---- aT / csT: [C_SH, D] -> [128, KT, C_SH] (bf16, PE transposes)
            aT = gp.tile([128, KT, C_SH], BF16, tag="aT")
            csT = gp.tile([128, KT, C_SH], BF16, tag="csT")
            for src, dst, nm in ((a_s, aT, "a"), (cs_s, csT, "c")):
                for g in range(2):
                    pst = ps_tr.tile([128, 4 * C_SH], BF16, tag="tr",
                                     name=f"pq{nm}{g}")
                    for j in range(4):
                        k = 4 * g + j
                        nc.tensor.transpose(pst[:, ts(j, C_SH)],
                                            src[:, ts(k, 128)],
                                            identb[0:C_SH, 0:C_SH])
                    nc.vector.tensor_copy(dst[:, ds(4 * g, 4), :].opt(), pst[:])

            # ---- scalar reductions: s_ac (rows 0:16), s_aa (32:48), s_cs (64:80)
            zpack = gp.tile([128, KT, 80], BF16, tag="zpack")
            nc.vector.tensor_mul(zpack[:, :, 0:C_SH].opt(), aT[:], csT[:])
            nc.vector.tensor_mul(zpack[:, :, 32:32 + C_SH].opt(), aT[:], aT[:])
            nc.vector.tensor_mul(zpack[:, :, 64:64 + C_SH].opt(), csT[:], csT[:])
            ps_sc = ps_acc.tile([80, 1], F32, tag="acc0")
            for k in range(KT):
                nc.tensor.matmul(ps_sc[:], zpack[:, k, :], ones_col[:],
                                 start=(k == 0), stop=(k == KT - 1))
            sc_s = sp.tile([80, 1], F32, tag="sc_s")
            nc.vector.tensor_copy(sc_s[:], ps_sc[:])
            sqq = sp.tile([C_SH, 1], F32, tag="sqq")
            nc.scalar.activation(sqq[:], sc_s[64:64 + C_SH, :], AF.Sqrt)
            shat = sp.tile([C_SH, 1], F32, tag="shat")
            nc.vector.reciprocal(shat[:], sqq[:])

            # ---- post-AllGather: bT + nb_row (all bf16)
            bfull = gp.tile([B_I, D], BF16, tag="bfull")
            nc.sync.dma_start(out=bfull[:], in_=ag_out[:, 0:D])
            nb_col = sp.tile([B_I, 1], BF16, tag="nb_col")
            nc.sync.dma_start(out=nb_col[:], in_=ag_out[:, D:D + 1])

            bT = gp.tile([128, KT, B_I], BF16, tag="bT")
            for g in range(2):
                pst = ps_tr.tile([128, 512], BF16, tag="tr", name=f"pb{g}")
                for j in range(4):
                    k = 4 * g + j
                    nc.tensor.transpose(pst[:, ts(j, 128)],
                                        bfull[:, ts(k, 128)], identb[:])
                nc.vector.tensor_copy(bT[:, ds(4 * g, 4), :].opt(), pst[:])

            ps_nbt = ps_tr.tile([1, 512], BF16, tag="tr", name="ps_nbt")
            nc.tensor.transpose(ps_nbt[:, 0:128], nb_col[:], identb[:])
            nb_row = sp.tile([1, B_I], BF16, tag="nb_row")
            nc.vector.tensor_copy(nb_row[:], ps_nbt[:, 0:128])

            # ---- similarity assembly
            ps_g1 = ps_acc.tile([C_SH, B_I], F32, tag="acc1")
            for k in range(KT):
                nc.tensor.matmul(ps_g1[:], aT[:, k, :], bT[:, k, :],
                                 start=(k == 0), stop=False)
            nc.tensor.matmul(ps_g1[:], ones_row[:, 0:C_SH], nb_row[:, :],
                             start=False, stop=True)
            den = sp.tile([C_SH, B_I], F32, tag="den")
            nc.scalar.activation(den[:], ps_g1[:], AF.Sqrt, scale=2.0,
                                 bias=sc_s[32:32 + C_SH, :])
            rden = sp.tile([C_SH, B_I], F32, tag="rden")
            nc.vector.reciprocal(rden[:], den[:])

            ps_g2 = ps_acc.tile([C_SH, B_I], F32, tag="acc2")
            for k in range(KT):
                nc.tensor.matmul(ps_g2[:], csT[:, k, :], bT[:, k, :],
                                 start=(k == 0), stop=(k == KT - 1))
            num = sp.tile([C_SH, B_I], F32, tag="num")
            nc.vector.tensor_scalar(
                out=num[:], in0=ps_g2[:], scalar1=sc_s[0:C_SH, :],
                scalar2=shat[:], op0=mybir.AluOpType.add,
                op1=mybir.AluOpType.mult)
            sims_s = sp.tile([C_SH, B_I], F32, tag="sims_s")
            nc.vector.tensor_mul(sims_s[:], num[:], rden[:])
            nc.sync.dma_start(out=sims_d[:, :], in_=sims_s[:])

    nc.compile()
    return nc


def _get_nc(CT: int, with_bias: bool):
    key = (CT, with_bias)
    if key not in _CACHE:
        _CACHE[key] = _build(CT, with_bias)
    return _CACHE[key]


def _balance_captions(lens):
    """Assign 16 captions to each of 8 cores, minimizing the max token sum
    (greedy LPT with per-core cardinality cap). Returns [8][C_SH] index array."""
    order = np.argsort(-lens, kind="stable")
    sums = np.zeros(N_CORES, np.int64)
    counts = np.zeros(N_CORES, np.int64)
    assign = [[] for _ in range(N_CORES)]
    for idx in order:
        open_cores = [m for m in range(N_CORES) if counts[m] < C_SH]
        m = min(open_cores, key=lambda m: (sums[m], m))
        assign[m].append(int(idx))
        sums[m] += int(lens[idx])
        counts[m] += 1
    return np.array(assign, np.int64)


def _host_prep(inputs):
    bf = ml_dtypes.bfloat16
    cap_embed = np.asarray(inputs["cap_embed"], dtype=np.float32)
    img_embed = np.asarray(inputs["img_embed"], dtype=np.float32)
    lens = np.asarray(inputs["lens"]).astype(np.int64)
    wvt = np.asarray(inputs["Wvt"], dtype=np.float32)
    wvi = np.asarray(inputs["Wvi"], dtype=np.float32)
    bvt = np.asarray(inputs["bvt"], dtype=np.float32).reshape(1, D)
    bvi = np.asarray(inputs["bvi"], dtype=np.float32).reshape(1, D)
    with_bias = bool(bvt.any() or bvi.any())
    gamma = float(np.asarray(inputs["gamma_img"]).reshape(-1)[0])

    assign = _balance_captions(lens)
    max_tok = int(lens[assign].sum(axis=1).max())
    CT = max(1, -(-max_tok // 128))
    CAP_TOK = CT * 128

    wvtT = np.ascontiguousarray(wvt.T.astype(bf))
    wviT = np.ascontiguousarray(wvi.T.astype(bf))
    om_b = np.zeros((IMG_PAD, I_SH), bf)
    om_b[:IMG_TOK] = np.repeat(np.eye(I_SH, dtype=bf), R, axis=0)
    identb = np.eye(128, dtype=bf)
    gam16 = np.full((C_SH, 1), gamma / R, np.float32)
    if with_bias:
        bias_vt = np.ascontiguousarray(np.repeat(bvt, 128, axis=0))
        bias_vi = np.ascontiguousarray(np.repeat(bvi, 128, axis=0))

    in_maps = []
    for m in range(N_CORES):
        idxs = assign[m]
        cap = np.zeros((CAP_TOK, D), np.float32)
        om_a = np.zeros((CAP_TOK, C_SH), bf)
        pos = 0
        for c, idx in enumerate(idxs):
            n = int(lens[idx])
            cap[pos:pos + n] = cap_embed[idx, :n]
            om_a[pos:pos + n, c] = 1.0
            pos += n
        img = np.zeros((IMG_PAD, D), np.float32)
        img[:IMG_TOK] = img_embed[m * I_SH:(m + 1) * I_SH].reshape(IMG_TOK, D)
        im = {
            "cap": np.ascontiguousarray(cap.astype(bf)),
            "capT": np.ascontiguousarray(cap.T.astype(bf)),
            "imgT": np.ascontiguousarray(img.T.astype(bf)),
            "wvtT": wvtT,
            "wviT": wviT,
            "om_a": om_a,
            "om_b": om_b,
            "idb": identb,
            "gam16": gam16,
        }
        if with_bias:
            im["bias_vt"] = bias_vt
            im["bias_vi"] = bias_vi
        in_maps.append(im)
    return in_maps, CT, with_bias, assign


def _unshard(res, assign):
    sims = np.empty((B_I, B_C), np.float32)
    for m in range(N_CORES):
        sims[:, assign[m]] = res.results[m]["sims"].T
    return sims


def kernel(**inputs) -> np.ndarray:
    in_maps, CT, with_bias, assign = _host_prep(inputs)
    nc = _get_nc(CT, with_bias)
    res = run_bass_kernel_spmd(nc, in_maps, core_ids=list(range(N_CORES)))
    return _unshard(res, assign)


def run_traced(**inputs):
    """For test.py: same as kernel() but with NTFF tracing enabled."""
    in_maps, CT, with_bias, assign = _host_prep(inputs)
    nc = _get_nc(CT, with_bias)
    res = run_bass_kernel_spmd(nc, in_maps, core_ids=list(range(N_CORES)),
                               trace=True)
    return _unshard(res, assign), res


# revision 6
# speedup vs baseline: 1.1125x; 1.1125x over previous
"""Trainium2 Bass kernel for nn_CrossAttn_65214783422649.

Algebraic reduction: softmax over R followed by mean over R is identically 1/R,
so the attention branch (Wq, Wk, energy, softmax) cancels:

    sims[i, c] = (a_c + b_i) . cs_c / (||a_c + b_i|| * ||cs_c||)
      a_c  = (gamma/R) * sum_t mask * leaky(cap_c @ Wvt.T + bvt)
      b_i  = mean_r leaky(img_i @ Wvi.T + bvi)
      cs_c = masked-sum_t cap_c          (the /lens cancels inside l2norm)

Sharding: captions 8-way (binpacked by lens so the packed-token tile count is
minimal), images 8-way for b, with a bf16 AllGather of b shards (+ 0.5|b|^2).

All transposed operands (imgT, capT, WviT, WvtT) are prepared host-side (no
device transposes outside the tiny assembly).  Matmul operands are bf16 (half
DMA, full PE rate, FWL, no fp32 LOW_HIGH double-pumping); PSUM accumulation is
fp32.  The img phase is ordered strictly before the caption phase on the PE so
the AllGather triggers as early as possible and overlaps the caption phase.
"""

import numpy as np
import ml_dtypes

import concourse.bass as bass
import concourse.mybir as mybir
import concourse.tile as tile
from concourse import bacc
from concourse.bass import ds, ts
from concourse.bass_utils import run_bass_kernel_spmd
from concourse.tile import add_dep_helper

F32 = mybir.dt.float32
BF16 = mybir.dt.bfloat16
AF = mybir.ActivationFunctionType

N_CORES = 8
B_I, B_C, R, T, D = 128, 128, 36, 64, 1024
C_SH = B_C // N_CORES          # 16 captions per core
I_SH = B_I // N_CORES          # 16 images per core
IMG_TOK = I_SH * R             # 576 image tokens per core
IMG_PAD = 640                  # padded to 5 * 128
KT = D // 128                  # 8 contraction tiles
IT = IMG_PAD // 128            # 5 image token tiles
NEG_SLOPE = 0.1
AGW = D + 1                    # AllGather row width: b row + 0.5*|b|^2

_CACHE: dict = {}


def _build(CT: int, with_bias: bool):
    """CT = number of 128-token caption tiles after host packing."""
    CAP_TOK = CT * 128
    nc = bacc.Bacc("TRN2", target_bir_lowering=False, debug=False,
                   num_devices=N_CORES)

    imgT_d = nc.dram_tensor("imgT", [D, IMG_PAD], BF16, kind="ExternalInput")
    wviT_d = nc.dram_tensor("wviT", [D, D], BF16, kind="ExternalInput")
    capT_d = nc.dram_tensor("capT", [D, CAP_TOK], BF16, kind="ExternalInput")
    wvtT_d = nc.dram_tensor("wvtT", [D, D], BF16, kind="ExternalInput")
    cap_d = nc.dram_tensor("cap", [CAP_TOK, D], BF16, kind="ExternalInput")
    om_a_d = nc.dram_tensor("om_a", [CAP_TOK, C_SH], BF16, kind="ExternalInput")
    om_b_d = nc.dram_tensor("om_b", [IMG_PAD, I_SH], BF16, kind="ExternalInput")
    idb_d = nc.dram_tensor("idb", [128, 128], BF16, kind="ExternalInput")
    gam_d = nc.dram_tensor("gam16", [C_SH, 1], F32, kind="ExternalInput")
    if with_bias:
        bft_d = nc.dram_tensor("bias_vt", [128, D], F32, kind="ExternalInput")
        bfi_d = nc.dram_tensor("bias_vi", [128, D], F32, kind="ExternalInput")
    sims_d = nc.dram_tensor("sims", [C_SH, B_I], F32, kind="ExternalOutput")

    with tile.TileContext(nc) as tc:
        with (
            tc.tile_pool(name="const", bufs=1) as const,
            tc.tile_pool(name="wt", bufs=1) as wtp,
            tc.tile_pool(name="xt", bufs=1) as xtp,
            tc.tile_pool(name="vtx", bufs=3) as vtxp,
            tc.tile_pool(name="gpool", bufs=1) as gp,
            tc.tile_pool(name="small", bufs=1) as sp,
            tc.tile_pool(name="ps_tr", bufs=1, space="PSUM") as ps_tr,
            tc.tile_pool(name="ps_mm", bufs=3, space="PSUM") as ps_mm,
            tc.tile_pool(name="ps_acc", bufs=1, space="PSUM") as ps_acc,
            tc.tile_pool(name="dram", bufs=1, space="DRAM") as dram,
        ):
            # ---- img-phase loads (first: the img phase gates the AllGather)
            om_b_s = const.tile([128, IT, I_SH], BF16, tag="om_b")
            nc.sync.dma_start(
                out=om_b_s[:], in_=om_b_d.rearrange("(a p) c -> p a c", p=128))
            imgT_s = xtp.tile([128, KT, IMG_PAD], BF16, tag="imgT")
            wvi_s = wtp.tile([128, KT, D], BF16, tag="wt_vi")
            for k in range(KT):
                nc.sync.dma_start(out=imgT_s[:, k, :], in_=imgT_d[ts(k, 128), :])
                nc.scalar.dma_start(out=wvi_s[:, k, :],
                                    in_=wviT_d[ts(k, 128), :])
            if with_bias:
                bias_vi = const.tile([128, D], F32, tag="bias_vi")
                nc.sync.dma_start(out=bias_vi[:], in_=bfi_d[:, :])
                bias_vt = const.tile([128, D], F32, tag="bias_vt")
                nc.sync.dma_start(out=bias_vt[:], in_=bft_d[:, :])

            identb = const.tile([128, 128], BF16, tag="idb")
            nc.sync.dma_start(out=identb[:], in_=idb_d[:, :])
            gam16 = const.tile([C_SH, 1], F32, tag="gam16")
            nc.sync.dma_start(out=gam16[:], in_=gam_d[:, :])
            ones_row = const.tile([1, 128], BF16, tag="ones_row")
            nc.vector.memset(ones_row[:], 1.0)
            ones_col = const.tile([128, 1], BF16, tag="ones_col")
            nc.vector.memset(ones_col[:], 1.0)

            # ---- img matmul phase -> b shard + 0.5*|b|^2, AllGather
            ps_b = [ps_acc.tile([I_SH, 512], F32, tag=f"acc{dh}", name=f"ps_b{dh}")
                    for dh in range(2)]
            for it in range(IT):
                vimg = vtxp.tile([128, D], BF16, tag="vtx", name=f"vimg{it}")
                for dh in range(2):
                    pm = ps_mm.tile([128, 512], F32, tag="mm", name=f"pmi{it}{dh}")
                    for k in range(KT):
                        nc.tensor.matmul(pm[:], imgT_s[:, k, ts(it, 128)],
                                         wvi_s[:, k, ds(dh * 512, 512)],
                                         start=(k == 0), stop=(k == KT - 1))
                    if with_bias:
                        nc.vector.tensor_add(pm[:], pm[:],
                                             bias_vi[:, ds(dh * 512, 512)])
                    nc.scalar.activation(vimg[:, ds(dh * 512, 512)], pm[:],
                                         AF.Prelu, alpha=NEG_SLOPE)
                    nc.tensor.matmul(ps_b[dh][:], om_b_s[:, it, :],
                                     vimg[:, ds(dh * 512, 512)],
                                     start=(it == 0), stop=(it == IT - 1))
            bnat = sp.tile([I_SH, AGW], F32, tag="bnat")
            for dh in range(2):
                nc.scalar.activation(bnat[:, ds(dh * 512, 512)], ps_b[dh][:],
                                     AF.Identity, scale=1.0 / R)
            bsq_sh = sp.tile([I_SH, D], F32, tag="bsq_sh")
            nc.vector.tensor_mul(bsq_sh[:], bnat[:, 0:D], bnat[:, 0:D])
            nbcol_l = sp.tile([I_SH, 1], F32, tag="nbcol_l")
            nc.vector.reduce_sum(nbcol_l[:], bsq_sh[:], axis=mybir.AxisListType.X)
            nc.vector.tensor_scalar_mul(bnat[:, D:D + 1], nbcol_l[:], 0.5)
            ag_in = dram.tile([I_SH, AGW], F32, tag="ag_in")
            ag_out = dram.tile([B_I, AGW], F32, addr_space="Shared", tag="ag_out")
            ag_dma = nc.sync.dma_start(out=ag_in[:], in_=bnat[:])
            nc.gpsimd.collective_compute(
                "AllGather",
                mybir.AluOpType.bypass,
                replica_groups=[list(range(N_CORES))],
                ins=[ag_in[:].opt()],
                outs=[ag_out[:].opt()],
            )

            # ---- cap-phase loads (issued after img loads; overlap img compute)
            om_a_s = const.tile([128, CT, C_SH], BF16, tag="om_a")
            nc.sync.dma_start(
                out=om_a_s[:], in_=om_a_d.rearrange("(a p) c -> p a c", p=128))
            capT_s = xtp.tile([128, KT, CAP_TOK], BF16, tag="capT")
            wvt_s = wtp.tile([128, KT, D], BF16, tag="wt_vt")
            for k in range(KT):
                nc.sync.dma_start(out=capT_s[:, k, :], in_=capT_d[ts(k, 128), :])
                nc.scalar.dma_start(out=wvt_s[:, k, :],
                                    in_=wvtT_d[ts(k, 128), :])
            cnats = []
            for ct in range(CT):
                cnat = xtp.tile([128, D], BF16, tag=f"cap{ct}", name=f"cnat{ct}")
                nc.sync.dma_start(out=cnat[:], in_=cap_d[ts(ct, 128), :])
                cnats.append(cnat)

            # ---- cap matmul phase -> a, capsum
            ps_a = [ps_acc.tile([C_SH, 512], F32, tag=f"acc{dh}",
                                name=f"ps_a{dh}")[:] for dh in range(2)]
            ps_cm = [ps_acc.tile([C_SH, 512], F32, tag=f"acc{dh+2}",
                                 name=f"ps_cm{dh}")[:] for dh in range(2)]
            first_cap_mm = None
            first_cap_act = None
            for ct in range(CT):
                vtxt = vtxp.tile([128, D], BF16, tag="vtx", name=f"vtxt{ct}")
                for dh in range(2):
                    pm = ps_mm.tile([128, 512], F32, tag="mm", name=f"pmc{ct}{dh}")
                    for k in range(KT):
                        mm = nc.tensor.matmul(pm[:], capT_s[:, k, ts(ct, 128)],
                                              wvt_s[:, k, ds(dh * 512, 512)],
                                              start=(k == 0), stop=(k == KT - 1))
                        if first_cap_mm is None:
                            first_cap_mm = mm
                    if with_bias:
                        nc.vector.tensor_add(pm[:], pm[:],
                                             bias_vt[:, ds(dh * 512, 512)])
                    act = nc.scalar.activation(vtxt[:, ds(dh * 512, 512)], pm[:],
                                               AF.Prelu, alpha=NEG_SLOPE)
                    if first_cap_act is None:
                        first_cap_act = act
                    nc.tensor.matmul(ps_a[dh], om_a_s[:, ct, :],
                                     vtxt[:, ds(dh * 512, 512)],
                                     start=(ct == 0), stop=(ct == CT - 1))
                    nc.tensor.matmul(ps_cm[dh], om_a_s[:, ct, :],
                                     cnats[ct][:, ds(dh * 512, 512)],
                                     start=(ct == 0), stop=(ct == CT - 1))
            # schedule the whole img phase + AllGather kickoff before cap work
            add_dep_helper(first_cap_mm.ins, ag_dma.ins, sync=False,
                           reason="cap matmuls after AllGather kickoff")
            add_dep_helper(first_cap_act.ins, ag_dma.ins, sync=False,
                           reason="cap evacs after AllGather kickoff")

            a_s = sp.tile([C_SH, D], BF16, tag="a_s")
            cs_s = sp.tile([C_SH, D], BF16, tag="cs_s")
            for dh in range(2):
                nc.scalar.activation(a_s[:, ds(dh * 512, 512)], ps_a[dh],
                                     AF.Identity, scale=gam16[:])
                nc.scalar.activation(cs_s[:, ds(dh * 512, 512)], ps_cm[dh],
                                     AF.Copy)

            # ---- aT / csT: [C_SH, D] -> [128, KT, C_SH] (bf16, PE transposes)
            aT = gp.tile([128, KT, C_SH], BF16, tag="aT")
            csT = gp.tile([128, KT, C_SH], BF16, tag="csT")
            for src, dst, nm in ((a_s, aT, "a"), (cs_s, csT, "c")):
                for g in range(2):
                    pst = ps_tr.tile([128, 4 * C_SH], BF16, tag="tr",
                                     name=f"pq{nm}{g}")
                    for j in range(4):
                        k = 4 * g + j
                        nc.tensor.transpose(pst[:, ts(j, C_SH)],
                                            src[:, ts(k, 128)],
                                            identb[0:C_SH, 0:C_SH])
                    nc.vector.tensor_copy(dst[:, ds(4 * g, 4), :].opt(), pst[:])

            # ---- scalar reductions: s_ac (rows 0:16), s_aa (32:48), s_cs (64:80)
            zpack = gp.tile([128, KT, 80], BF16, tag="zpack")
            nc.vector.tensor_mul(zpack[:, :, 0:C_SH].opt(), aT[:], csT[:])
            nc.vector.tensor_mul(zpack[:, :, 32:32 + C_SH].opt(), aT[:], aT[:])
            nc.vector.tensor_mul(zpack[:, :, 64:64 + C_SH].opt(), csT[:], csT[:])
            ps_sc = ps_acc.tile([80, 1], F32, tag="acc0")
            for k in range(KT):
                nc.tensor.matmul(ps_sc[:], zpack[:, k, :], ones_col[:],
                                 start=(k == 0), stop=(k == KT - 1))
            sc_s = sp.tile([80, 1], F32, tag="sc_s")
            nc.vector.tensor_copy(sc_s[:], ps_sc[:])
            sqq = sp.tile([C_SH, 1], F32, tag="sqq")
            nc.scalar.activation(sqq[:], sc_s[64:64 + C_SH, :], AF.Sqrt)
            shat = sp.tile([C_SH, 1], F32, tag="shat")
            nc.vector.reciprocal(shat[:], sqq[:])

            # ---- post-AllGather: bT + nb_row (all bf16)
            bfull = gp.tile([B_I, D], BF16, tag="bfull")
            nc.gpsimd.dma_start(out=bfull[:], in_=ag_out[:, 0:D])
            nb_col = sp.tile([B_I, 1], BF16, tag="nb_col")
            nc.gpsimd.dma_start(out=nb_col[:], in_=ag_out[:, D:D + 1])

            bT = gp.tile([128, KT, B_I], BF16, tag="bT")
            for g in range(2):
                pst = ps_tr.tile([128, 512], BF16, tag="tr", name=f"pb{g}")
                for j in range(4):
                    k = 4 * g + j
                    nc.tensor.transpose(pst[:, ts(j, 128)],
                                        bfull[:, ts(k, 128)], identb[:])
                nc.vector.tensor_copy(bT[:, ds(4 * g, 4), :].opt(), pst[:])

            ps_nbt = ps_tr.tile([1, 512], BF16, tag="tr", name="ps_nbt")
            nc.tensor.transpose(ps_nbt[:, 0:128], nb_col[:], identb[:])
            nb_row = sp.tile([1, B_I], BF16, tag="nb_row")
            nc.vector.tensor_copy(nb_row[:], ps_nbt[:, 0:128])

            # ---- similarity assembly
            ps_g1 = ps_acc.tile([C_SH, B_I], F32, tag="acc1")
            for k in range(KT):
                nc.tensor.matmul(ps_g1[:], aT[:, k, :], bT[:, k, :],
                                 start=(k == 0), stop=False)
            nc.tensor.matmul(ps_g1[:], ones_row[:, 0:C_SH], nb_row[:, :],
                             start=False, stop=True)
            den = sp.tile([C_SH, B_I], F32, tag="den")
            nc.scalar.activation(den[:], ps_g1[:], AF.Sqrt, scale=2.0,
                                 bias=sc_s[32:32 + C_SH, :])
            rden = sp.tile([C_SH, B_I], F32, tag="rden")
            nc.vector.reciprocal(rden[:], den[:])

            ps_g2 = ps_acc.tile([C_SH, B_I], F32, tag="acc2")
            for k in range(KT):
                nc.tensor.matmul(ps_g2[:], csT[:, k, :], bT[:, k, :],
                                 start=(k == 0), stop=(k == KT - 1))
            num = sp.tile([C_SH, B_I], F32, tag="num")
            nc.vector.tensor_scalar(
                out=num[:], in0=ps_g2[:], scalar1=sc_s[0:C_SH, :],
                scalar2=shat[:], op0=mybir.AluOpType.add,
                op1=mybir.AluOpType.mult)
            sims_s = sp.tile([C_SH, B_I], F32, tag="sims_s")
            nc.vector.tensor_mul(sims_s[:], num[:], rden[:])
            nc.sync.dma_start(out=sims_d[:, :], in_=sims_s[:])

    nc.compile()
    return nc


def _get_nc(CT: int, with_bias: bool):
    key = (CT, with_bias)
    if key not in _CACHE:
        _CACHE[key] = _build(CT, with_bias)
    return _CACHE[key]


def _balance_captions(lens):
    """Assign 16 captions to each of 8 cores, minimizing the max token sum
    (greedy LPT with per-core cardinality cap). Returns [8][C_SH] index array."""
    order = np.argsort(-lens, kind="stable")
    sums = np.zeros(N_CORES, np.int64)
    counts = np.zeros(N_CORES, np.int64)
    assign = [[] for _ in range(N_CORES)]
    for idx in order:
        open_cores = [m for m in range(N_CORES) if counts[m] < C_SH]
        m = min(open_cores, key=lambda m: (sums[m], m))
        assign[m].append(int(idx))
        sums[m] += int(lens[idx])
        counts[m] += 1
    return np.array(assign, np.int64)


def _host_prep(inputs):
    bf = ml_dtypes.bfloat16
    cap_embed = np.asarray(inputs["cap_embed"], dtype=np.float32)
    img_embed = np.asarray(inputs["img_embed"], dtype=np.float32)
    lens = np.asarray(inputs["lens"]).astype(np.int64)
    wvt = np.asarray(inputs["Wvt"], dtype=np.float32)
    wvi = np.asarray(inputs["Wvi"], dtype=np.float32)
    bvt = np.asarray(inputs["bvt"], dtype=np.float32).reshape(1, D)
    bvi = np.asarray(inputs["bvi"], dtype=np.float32).reshape(1, D)
    with_bias = bool(bvt.any() or bvi.any())
    gamma = float(np.asarray(inputs["gamma_img"]).reshape(-1)[0])

    assign = _balance_captions(lens)
    max_tok = int(lens[assign].sum(axis=1).max())
    CT = max(1, -(-max_tok // 128))
    CAP_TOK = CT * 128

    wvtT = np.ascontiguousarray(wvt.T.astype(bf))
    wviT = np.ascontiguousarray(wvi.T.astype(bf))
    om_b = np.zeros((IMG_PAD, I_SH), bf)
    om_b[:IMG_TOK] = np.repeat(np.eye(I_SH, dtype=bf), R, axis=0)
    identb = np.eye(128, dtype=bf)
    gam16 = np.full((C_SH, 1), gamma / R, np.float32)
    if with_bias:
        bias_vt = np.ascontiguousarray(np.repeat(bvt, 128, axis=0))
        bias_vi = np.ascontiguousarray(np.repeat(bvi, 128, axis=0))

    in_maps = []
    for m in range(N_CORES):
        idxs = assign[m]
        cap = np.zeros((CAP_TOK, D), np.float32)
        om_a = np.zeros((CAP_TOK, C_SH), bf)
        pos = 0
        for c, idx in enumerate(idxs):
            n = int(lens[idx])
            cap[pos:pos + n] = cap_embed[idx, :n]
            om_a[pos:pos + n, c] = 1.0
            pos += n
        img = np.zeros((IMG_PAD, D), np.float32)
        img[:IMG_TOK] = img_embed[m * I_SH:(m + 1) * I_SH].reshape(IMG_TOK, D)
        im = {
            "cap": np.ascontiguousarray(cap.astype(bf)),
            "capT": np.ascontiguousarray(cap.T.astype(bf)),
            "imgT": np.ascontiguousarray(img.T.astype(bf)),
            "wvtT": wvtT,
            "wviT": wviT,
            "om_a": om_a,
            "om_b": om_b,
            "idb": identb,
            "gam16": gam16,
        }
        if with_bias:
            im["bias_vt"] = bias_vt
            im["bias_vi"] = bias_vi
        in_maps.append(im)
    return in_maps, CT, with_bias, assign


def _unshard(res, assign):
    sims = np.empty((B_I, B_C), np.float32)
    for m in range(N_CORES):
        sims[:, assign[m]] = res.results[m]["sims"].T
    return sims


def kernel(**inputs) -> np.ndarray:
    in_maps, CT, with_bias, assign = _host_prep(inputs)
    nc = _get_nc(CT, with_bias)
    res = run_bass_kernel_spmd(nc, in_maps, core_ids=list(range(N_CORES)))
    return _unshard(res, assign)


def run_traced(**inputs):
    """For test.py: same as kernel() but with NTFF tracing enabled."""
    in_maps, CT, with_bias, assign = _host_prep(inputs)
    nc = _get_nc(CT, with_bias)
    res = run_bass_kernel_spmd(nc, in_maps, core_ids=list(range(N_CORES)),
                               trace=True)
    return _unshard(res, assign), res
